# revision 37
# baseline (speedup 1.0000x reference)
"""GAT layer kernel for Trainium2, distributed over 8 NeuronCores.

Reference computation (per graph-attention layer):
    h = x @ W                                   [n, d]
    e = (h@a1)[:,None] + (h@a2)[None,:] + b     [n, n]
    e = leaky_relu(e, 0.2)
    e = where(adj == 0, -inf, e)
    alpha = softmax(e, axis=1)
    alpha *= exp(-dist) * (clip(cos(angle), 0) + 1e-6)
    alpha /= sum(alpha, axis=1)
    out = alpha @ h                             [n, d]

Distribution: each core owns a 512-row block of the [n, n] attention
matrix.  The softmax normalizer cancels against the final renorm, so the
unnormalized weight is
    w = exp(leaky(e) + L),   L = -dist + log(clip(cos(angle),0)+1e-6)
with L := -20000 on masked (adj==0) entries so exp underflows to exactly
0.  L is input-only data, so the host folds dist/angle/adj into ONE fp16
matrix streamed per core (4 MiB instead of 16 MiB) and the device-side
physics work collapses to one DVE add.

On-chip layout puts j (columns) on partitions and i (rows) on the free
dim, so the final contraction w.T-block @ [h | 1] runs natively on the
tensor engine (fp16 operands) and row sums fall out of the ones column.
Per j-tile pipeline:  DMA L16 -> PE rank-4 e-matmul -> ACT Prelu
(PSUM->SBUF fp16) -> DVE +L -> ACT Exp -> PE contraction.
"""

import numpy as np

import concourse.bass as bass
import concourse.bacc as bacc
import concourse.mybir as mybir
import concourse.tile as tile

N = 4096
DIM = 128
NCORES = 8
R = N // NCORES          # rows per core (512)
PJ = 128                 # j per partition tile
NJT = N // PJ            # 32 j-tiles
NEG_SLOPE = 0.2
MASKL = -2.0e4           # additive log-mask; exp -> exactly 0 (fp16-safe)
F32 = mybir.dt.float32
F16 = mybir.dt.float16
AF = mybir.ActivationFunctionType
ALU = mybir.AluOpType
PSUM = bass.MemorySpace.PSUM


def build_nc(n=N, dim=DIM, r=R, grp=2, repeat=1, chop=1, abl=frozenset(),
             ebufs=2, ubufs=3, dsup=4, dbufs=2, pack2=0, dvp=0, pooladd=0):
    """Build the per-core Bass program (identical on every core).

    grp:  j-tiles fused per elementwise op (FD = grp*512)
    dsup: elementwise groups per L-stream DMA (1 MiB batches at dsup=4)
    chop: split the DVE add into this many free-dim chunks
    pack2: run e-matmul pairs concurrently in 32-row PE strips
          (tile_position rows 0/32) — halves their PE occupancy
    dvp:  of every ngrp elementwise groups, run this many groups' leaky
          on DVE (2 extra DVE ops) instead of ACT Prelu — balances the
          two engines (ACT is otherwise the saturated bottleneck)
    abl:  ablation flags ("noprelu", "noadd", "noexp", "nomm", "nodma")
    """
    njt = n // PJ
    ngrp = njt // grp
    fr = grp * r                 # free elems per group op
    nib = r // PJ                # i sub-blocks per core (4)

    nc = bacc.Bacc("TRN2", target_bir_lowering=False, debug=False)

    xT = nc.dram_tensor("xT", [dim, n], F32, kind="ExternalInput")
    xTb = nc.dram_tensor("xTb", [dim, r], F32, kind="ExternalInput")
    W = nc.dram_tensor("W", [dim, dim], F32, kind="ExternalInput")
    w1 = nc.dram_tensor("w1", [dim, 1], F32, kind="ExternalInput")
    w2 = nc.dram_tensor("w2", [dim, 1], F32, kind="ExternalInput")
    b128 = nc.dram_tensor("b128", [PJ, 1], F32, kind="ExternalInput")
    ones2h = nc.dram_tensor("ones2h", [2, n], F16, kind="ExternalInput")
    # L^T marshaled at j-tile-PAIR granularity: row block q*128+p holds
    # j-tiles (2q, 2q+1) as a contiguous [2, r] line -> 2 KiB per
    # partition per DMA descriptor row, independent of grp/dsup.
    Lm = nc.dram_tensor("Lm", [(njt // 2) * PJ, 2 * r], F16,
                        kind="ExternalInput")
    out = nc.dram_tensor("out", [r, dim], F32, kind="ExternalOutput")
    thl_dram = nc.dram_tensor("thl_dram", [2, n], F16)
    shl_dram = nc.dram_tensor("shl_dram", [2, r], F16)

    with tile.TileContext(nc) as tc:
        # ---------- long-lived tensors ----------
        cpool = tc.alloc_tile_pool(name="const", bufs=1)
        h_sb = cpool.tile([PJ, njt, dim + 1], F16, tag="h")  # [h | 1]
        nc.vector.memset(h_sb[:, :, dim:dim + 1], 1.0)
        if pack2:
            # Paired K=4 operands in PE row strips 0-3 / 32-35: strip 0
            # holds even j-tiles' [t_hi, t_lo, 1, 1], strip 32 odd ones.
            t4_sb = cpool.tile([36, n // 2], F16, tag="t4")
            s4_sb = cpool.tile([36, r], F16, tag="s4")
            nc.sync.dma_start(t4_sb[2:4, :], ones2h[:, 0:n // 2])
            nc.sync.dma_start(t4_sb[34:36, :], ones2h[:, 0:n // 2])
            nc.sync.dma_start(s4_sb[0:2, :], ones2h[:, 0:r])
            nc.sync.dma_start(s4_sb[32:34, :], ones2h[:, 0:r])
        else:
            # K=4 fp16 hi/lo rank-2 operands: e = (t_hi+t_lo)+(s_hi+s_lo)
            t4_sb = cpool.tile([4, n], F16, tag="t4")   # t_hi, t_lo, 1, 1
            s4_sb = cpool.tile([4, r], F16, tag="s4")   # 1, 1, s_hi, s_lo
            nc.sync.dma_start(t4_sb[2:4, :], ones2h[:])
            nc.sync.dma_start(s4_sb[0:2, :], ones2h[:, 0:r])

        # ---------- prologue: h = x@W, t = x@w2+b, s = x@w1 ----------
        plpool = tc.alloc_tile_pool(name="prolsb", bufs=1)
        ppool = tc.alloc_tile_pool(name="prolps", bufs=2, space=PSUM)

        xT_sb = plpool.tile([dim, n], F32, tag="xT")
        nc.sync.dma_start(xT_sb[:], xT[:])
        xTb_sb = plpool.tile([dim, r], F32, tag="xTb")
        nc.sync.dma_start(xTb_sb[:], xTb[:])
        w1_sb = plpool.tile([dim, 1], F32, tag="w1")
        nc.sync.dma_start(w1_sb[:], w1[:])
        b128_sb = plpool.tile([PJ, 1], F32, tag="b128")
        nc.sync.dma_start(b128_sb[:], b128[:])
        # Fused prologue: one matmul per j-tile with rhs = [W | w2]
        # yields the h tile AND the t column (t[j] = x[j]@w2) for free;
        # t lands column-major as t128[p, jt] = t[jt*128 + p].
        Wx_sb = plpool.tile([dim, dim + 1], F32, tag="Wx")
        nc.sync.dma_start(Wx_sb[:, 0:dim], W[:])
        nc.sync.dma_start(Wx_sb[:, dim:dim + 1], w2[:])
        t128 = plpool.tile([PJ, njt], F32, tag="t128")
        for jt in range(njt):
            hp = ppool.tile([PJ, dim + 1], F32, tag="hp", name=f"hp{jt}")
            nc.tensor.matmul(hp[:], xT_sb[:, jt * PJ:(jt + 1) * PJ], Wx_sb[:])
            nc.vector.tensor_copy(h_sb[:, jt, 0:dim], hp[:, 0:dim])
            nc.vector.tensor_copy(t128[:, jt:jt + 1], hp[:, dim:dim + 1])
        s128 = plpool.tile([PJ, nib], F32, tag="s128")
        for c in range(nib):
            sp = ppool.tile([PJ, 1], F32, tag="sp", name=f"sp{c}")
            nc.tensor.matmul(sp[:], xTb_sb[:, c * PJ:(c + 1) * PJ], w1_sb[:])
            nc.vector.tensor_copy(s128[:, c:c + 1], sp[:])

        def hilo(r128, hl_dram, dst_rows, pfx, bias=None, dst2=None):
            # hi/lo fp16 split on all 128 DVE lanes; j = c*128 + p
            hi = plpool.tile(list(r128.shape), F16, tag=f"{pfx}hi")
            lo = plpool.tile(list(r128.shape), F16, tag=f"{pfx}lo")
            if bias is None:
                nc.vector.tensor_copy(hi[:], r128[:])
                nc.vector.scalar_tensor_tensor(
                    lo[:], r128[:], 1.0, hi[:], ALU.bypass, ALU.subtract)
            else:
                nc.vector.tensor_scalar_add(hi[:], r128[:], bias)
                nc.vector.scalar_tensor_tensor(
                    lo[:], r128[:], bias, hi[:], ALU.add, ALU.subtract)
            nc.sync.dma_start(
                hl_dram[0:1, :].rearrange("o (c p) -> (o p) c", p=PJ), hi[:])
            nc.sync.dma_start(
                hl_dram[1:2, :].rearrange("o (c p) -> (o p) c", p=PJ), lo[:])
            if dst2 is None:
                nc.sync.dma_start(dst_rows, hl_dram[:])
            else:
                for dst, src in dst2:
                    nc.sync.dma_start(dst, src)

        if pack2:
            # split thl by j-tile parity into the two PE row strips
            thl_par = thl_dram[:].rearrange("o (q t c) -> t o q c",
                                            t=2, c=PJ)
            t4e = t4_sb[0:2, :].rearrange("o (q c) -> o q c", c=PJ)
            t4o = t4_sb[32:34, :].rearrange("o (q c) -> o q c", c=PJ)
            hilo(t128[:], thl_dram, t4_sb[0:2, :], "t", bias=b128_sb[:],
                 dst2=[(t4e, thl_par[0]), (t4o, thl_par[1])])
            hilo(s128[:], shl_dram, s4_sb[2:4, :], "s",
                 dst2=[(s4_sb[2:4, :], shl_dram[:]),
                       (s4_sb[34:36, :], shl_dram[:])])
        else:
            hilo(t128[:], thl_dram, t4_sb[0:2, :], "t", bias=b128_sb[:])
            hilo(s128[:], shl_dram, s4_sb[2:4, :], "s")

        ppool.release()
        plpool.release()

        # ---------- main-loop pools ----------
        dpool = tc.alloc_tile_pool(name="dstream", bufs=dbufs)
        wpool = tc.alloc_tile_pool(name="work", bufs=2)
        upool = tc.alloc_tile_pool(name="uhold", bufs=ubufs)
        opool = tc.alloc_tile_pool(name="epi", bufs=4)
        accpool = tc.alloc_tile_pool(name="acc", bufs=1, space=PSUM)
        epool = tc.alloc_tile_pool(name="eps", bufs=ebufs, space=PSUM)

        nA = ngrp - dvp
        for rep in range(repeat):
            ia = 0  # index over flowA (ACT-prelu) groups, for pooladd
            acc = [accpool.tile([PJ, dim + 1], F32, tag=f"acc{ib}",
                                name=f"acc{rep}_{ib}")
                   for ib in range(nib)] if "nomm" not in abl else None
            for g in range(ngrp):
                lt = None
                if "nodma" not in abl:
                    if g % dsup == 0:
                        # one 1 MiB DMA covers dsup elementwise groups
                        npair = dsup * grp // 2
                        ltb = dpool.tile([PJ, npair, 2 * r], F16, tag="lt",
                                         name=f"lt{rep}_{g}")
                        q0 = g * grp // 2
                        nc.sync.dma_start(
                            ltb[:],
                            Lm[q0 * PJ:(q0 + npair) * PJ, :]
                            .rearrange("(q p) f -> p q f", p=PJ))
                    p0 = (g % dsup) * grp // 2
                    lt = (ltb[:, p0:p0 + grp // 2, :]
                          .rearrange("p a i -> p (a i)"))

                e_ps = epool.tile([PJ, grp, r], F32, tag="e",
                                  name=f"e{rep}_{g}")
                if pack2:
                    for a in range(0, grp, 2):
                        q = (g * grp + a) // 2
                        cs = slice(q * PJ, (q + 1) * PJ)
                        nc.tensor.matmul(e_ps[:, a, :], t4_sb[0:4, cs],
                                         s4_sb[0:4, :], tile_position=(0, 0))
                        nc.tensor.matmul(e_ps[:, a + 1, :], t4_sb[32:36, cs],
                                         s4_sb[32:36, :],
                                         tile_position=(32, 0))
                else:
                    for a in range(grp):
                        jt = g * grp + a
                        nc.tensor.matmul(e_ps[:, a, :],
                                         t4_sb[:, jt * PJ:(jt + 1) * PJ],
                                         s4_sb[:])
                epf = e_ps[:].rearrange("p a i -> p (a i)")

                # Ablations substitute inputs rather than skip consumers so
                # every tile that is read stays written (Tile requires it).
                dve_leaky = (g % ngrp) * dvp // ngrp != \
                    ((g % ngrp) + 1) * dvp // ngrp  # spread dvp-of-ngrp
                src0 = None
                if "noprelu" not in abl:
                    e2 = wpool.tile([PJ, fr], F16, tag="e2",
                                    name=f"e2{rep}_{g}")
                    if dve_leaky:
                        # leaky on DVE with ONE PSUM pass: drain 0.2e,
                        # then leaky = max(5*(0.2e), 0.2e) on SBUF fp16
                        # (both operands 16-bit -> 2x perf mode)
                        rt = wpool.tile([PJ, fr], F16, tag="rt",
                                        name=f"rt{rep}_{g}")
                        nc.vector.tensor_scalar_mul(rt[:], epf, 0.2)
                        nc.vector.scalar_tensor_tensor(
                            e2[:], rt[:], 5.0, rt[:], ALU.mult, ALU.max)
                    else:
                        # single ACT Prelu (HW-verified alpha semantics)
                        nc.scalar.activation(e2[:], epf, AF.Prelu,
                                             alpha=NEG_SLOPE)
                    src0 = e2[:]
                if src0 is None:
                    src0 = lt
                assert src0 is not None, "noprelu+nodma unsupported"
                src1 = lt if lt is not None else src0
                esrc = src0
                if "noadd" not in abl:
                    gt = wpool.tile([PJ, fr], F16, tag="g", name=f"g{rep}_{g}")
                    # offload some flowA adds to the otherwise-idle GPSIMD
                    on_pool = False
                    if not dve_leaky and nA > 0:
                        on_pool = ia * pooladd // nA != \
                            (ia + 1) * pooladd // nA
                        ia += 1
                    eng = nc.gpsimd if on_pool else nc.vector
                    cw = fr // chop
                    for cc in range(chop):
                        s = slice(cc * cw, (cc + 1) * cw)
                        eng.tensor_add(gt[:, s], src0[:, s], src1[:, s])
                    esrc = gt[:]
                if "noexp" not in abl:
                    ut = upool.tile([PJ, grp, r], F16, tag="u",
                                    name=f"u{rep}_{g}")
                    nc.scalar.activation(
                        ut[:].rearrange("p a i -> p (a i)"), esrc, AF.Exp)

                if "nomm" not in abl:
                    for a in range(grp):
                        jt = g * grp + a
                        for ib in range(nib):
                            w128 = (esrc[:, a * r + ib * PJ:
                                         a * r + (ib + 1) * PJ]
                                    if "noexp" in abl else
                                    ut[:, a, ib * PJ:(ib + 1) * PJ])
                            nc.tensor.matmul(
                                acc[ib][:], w128, h_sb[:, jt, :],
                                start=(jt == 0), stop=(jt == njt - 1))

            # ---------- epilogue: out = num / rowsum ----------
            # (reference's +1e-9 is ~1e-12 relative here: rowsums are the
            # UNnormalized exp-sums, O(100), so the epsilon is dropped)
            if "nomm" not in abl:
                ot4 = opool.tile([PJ, nib, dim], F32, tag="ot",
                                 name=f"ot{rep}")
                for ib in range(nib):
                    rec = opool.tile([PJ, 1], F32, tag="rec",
                                     name=f"rec{rep}_{ib}")
                    nc.vector.reciprocal(rec[:], acc[ib][:, dim:dim + 1])
                    nc.vector.tensor_scalar_mul(ot4[:, ib, :],
                                                acc[ib][:, 0:dim], rec[:])
                nc.sync.dma_start(
                    out[:].rearrange("(q p) d -> p q d", p=PJ), ot4[:])

        epool.release()
        accpool.release()
        opool.release()
        upool.release()
        wpool.release()
        dpool.release()
        cpool.release()

    nc.compile()
    return nc


_NC_CACHE = {}


def _get_nc(**kw):
    key = tuple(sorted((k, v) for k, v in kw.items()))
    if key not in _NC_CACHE:
        _NC_CACHE[key] = build_nc(**kw)
    return _NC_CACHE[key]


def host_prep(x, adj, dist_mat, angle_mat, W, attn_w, attn_b, n=N, dim=DIM,
              ncores=NCORES):
    """Shard + marshal inputs into the per-core layout."""
    x = np.ascontiguousarray(np.asarray(x, dtype=np.float32))
    adj = np.asarray(adj)
    dist_mat = np.asarray(dist_mat, dtype=np.float32)
    angle_mat = np.asarray(angle_mat, dtype=np.float32)
    W = np.ascontiguousarray(np.asarray(W, dtype=np.float32))
    attn_w = np.asarray(attn_w, dtype=np.float32)
    attn_b = np.asarray(attn_b, dtype=np.float32)

    r = n // ncores
    xT = np.ascontiguousarray(x.T)                      # [dim, n]
    w1 = np.ascontiguousarray((W @ attn_w[:dim]).reshape(dim, 1))
    w2 = np.ascontiguousarray((W @ attn_w[dim:]).reshape(dim, 1))
    bb = float(attn_b.reshape(-1)[0])

    # Fold the physics rescale + adjacency mask into one log-domain
    # matrix: w = exp(leaky(e) + L); masked entries underflow to 0.
    cosw = np.clip(np.cos(angle_mat), 0.0, None) + np.float32(1e-6)
    L = np.where(adj != 0, -dist_mat + np.log(cosw),
                 np.float32(MASKL)).astype(np.float32)

    in_maps = []
    njt = n // PJ
    for c in range(ncores):
        sl = slice(c * r, (c + 1) * r)
        LT = L[sl].T.astype(np.float16)                 # [n, r]
        # pair-granularity marshal: [njt/2, 2, 128, r] -> [njt/2, 128, 2, r]
        Lm = np.ascontiguousarray(
            LT.reshape(njt // 2, 2, PJ, r).transpose(0, 2, 1, 3)
            .reshape((njt // 2) * PJ, 2 * r))
        in_maps.append({
            "ones2h": np.ones((2, n), dtype=np.float16),
            "b128": np.full((PJ, 1), bb, dtype=np.float32),
            "xT": xT,
            "xTb": np.ascontiguousarray(xT[:, sl]),
            "W": W,
            "w1": w1,
            "w2": w2,
            "Lm": Lm,
        })
    return in_maps


def kernel(x, adj, dist_mat, angle_mat, W, attn_w, attn_b):
    from concourse.bass_utils import run_bass_kernel_spmd

    nc = _get_nc()
    in_maps = host_prep(x, adj, dist_mat, angle_mat, W, attn_w, attn_b)
    last_err = None
    for attempt in range(3):
        try:
            res = run_bass_kernel_spmd(nc, in_maps,
                                       core_ids=list(range(NCORES)))
            return np.concatenate(
                [res.results[c]["out"] for c in range(NCORES)], axis=0)
        except Exception as ex:  # axon terminals occasionally come up wedged
            last_err = ex
            try:
                import jax
                jax.clear_caches()
                jax._src.api.clear_backends()
            except Exception:
                pass
    raise last_err


# revision 44
# speedup vs baseline: 1.0233x; 1.0233x over previous
"""GAT layer kernel for Trainium2, distributed over 8 NeuronCores.

Reference computation (per graph-attention layer):
    h = x @ W                                   [n, d]
    e = (h@a1)[:,None] + (h@a2)[None,:] + b     [n, n]
    e = leaky_relu(e, 0.2)
    e = where(adj == 0, -inf, e)
    alpha = softmax(e, axis=1)
    alpha *= exp(-dist) * (clip(cos(angle), 0) + 1e-6)
    alpha /= sum(alpha, axis=1)
    out = alpha @ h                             [n, d]

Distribution: each core owns a 512-row block of the [n, n] attention
matrix.  The softmax normalizer cancels against the final renorm, so the
unnormalized weight is
    w = exp(leaky(e) + L),   L = -dist + log(clip(cos(angle),0)+1e-6)
with L := -20000 on masked (adj==0) entries so exp underflows to exactly
0.  L is input-only data, so the host folds dist/angle/adj into ONE fp16
matrix streamed per core (4 MiB instead of 16 MiB) and the device-side
physics work collapses to one DVE add.

On-chip layout puts j (columns) on partitions and i (rows) on the free
dim, so the final contraction w.T-block @ [h | 1] runs natively on the
tensor engine (fp16 operands) and row sums fall out of the ones column.
Per j-tile pipeline:  DMA L16 -> PE rank-4 e-matmul -> ACT Prelu
(PSUM->SBUF fp16) -> DVE +L -> ACT Exp -> PE contraction.
"""

import numpy as np

import concourse.bass as bass
import concourse.bacc as bacc
import concourse.mybir as mybir
import concourse.tile as tile

N = 4096
DIM = 128
NCORES = 8
R = N // NCORES          # rows per core (512)
PJ = 128                 # j per partition tile
NJT = N // PJ            # 32 j-tiles
NEG_SLOPE = 0.2
MASKL = -2.0e4           # additive log-mask; exp -> exactly 0 (fp16-safe)
F32 = mybir.dt.float32
F16 = mybir.dt.float16
AF = mybir.ActivationFunctionType
ALU = mybir.AluOpType
PSUM = bass.MemorySpace.PSUM


def build_nc(n=N, dim=DIM, r=R, grp=2, repeat=1, chop=1, abl=frozenset(),
             ebufs=2, ubufs=3, dsup=4, dbufs=2, pack2=0, dvp=0, pooladd=0,
             fuse2=1):
    """Build the per-core Bass program (identical on every core).

    grp:  j-tiles fused per elementwise op (FD = grp*512)
    dsup: elementwise groups per L-stream DMA (1 MiB batches at dsup=4)
    chop: split the DVE add into this many free-dim chunks
    pack2: run e-matmul pairs concurrently in 32-row PE strips
          (tile_position rows 0/32) — halves their PE occupancy
    dvp:  of every ngrp elementwise groups, run this many groups' leaky
          on DVE (2 extra DVE ops) instead of ACT Prelu — balances the
          two engines (ACT is otherwise the saturated bottleneck)
    abl:  ablation flags ("noprelu", "noadd", "noexp", "nomm", "nodma")
    """
    njt = n // PJ
    ngrp = njt // grp
    fr = grp * r                 # free elems per group op
    nib = r // PJ                # i sub-blocks per core (4)

    nc = bacc.Bacc("TRN2", target_bir_lowering=False, debug=False)

    xT = nc.dram_tensor("xT", [dim, n], F32, kind="ExternalInput")
    xTb = nc.dram_tensor("xTb", [dim, r], F32, kind="ExternalInput")
    W = nc.dram_tensor("W", [dim, dim], F32, kind="ExternalInput")
    w1 = nc.dram_tensor("w1", [dim, 1], F32, kind="ExternalInput")
    w2 = nc.dram_tensor("w2", [dim, 1], F32, kind="ExternalInput")
    b128 = nc.dram_tensor("b128", [PJ, 1], F32, kind="ExternalInput")
    ones2h = nc.dram_tensor("ones2h", [2, n], F16, kind="ExternalInput")
    # L^T marshaled at j-tile-PAIR granularity: row block q*128+p holds
    # j-tiles (2q, 2q+1) as a contiguous [2, r] line -> 2 KiB per
    # partition per DMA descriptor row, independent of grp/dsup.
    Lm = nc.dram_tensor("Lm", [(njt // 2) * PJ, 2 * r], F16,
                        kind="ExternalInput")
    out = nc.dram_tensor("out", [r, dim], F32, kind="ExternalOutput")
    thl_dram = nc.dram_tensor("thl_dram", [2, n], F16)
    shl_dram = nc.dram_tensor("shl_dram", [2, r], F16)

    with tile.TileContext(nc) as tc:
        # ---------- long-lived tensors ----------
        cpool = tc.alloc_tile_pool(name="const", bufs=1)
        h_sb = cpool.tile([PJ, njt, dim + 1], F16, tag="h")  # [h | 1]
        nc.vector.memset(h_sb[:, :, dim:dim + 1], 1.0)
        if pack2:
            # Paired K=4 operands in PE row strips 0-3 / 32-35: strip 0
            # holds even j-tiles' [t_hi, t_lo, 1, 1], strip 32 odd ones.
            t4_sb = cpool.tile([36, n // 2], F16, tag="t4")
            s4_sb = cpool.tile([36, r], F16, tag="s4")
            nc.sync.dma_start(t4_sb[2:4, :], ones2h[:, 0:n // 2])
            nc.sync.dma_start(t4_sb[34:36, :], ones2h[:, 0:n // 2])
            nc.sync.dma_start(s4_sb[0:2, :], ones2h[:, 0:r])
            nc.sync.dma_start(s4_sb[32:34, :], ones2h[:, 0:r])
        else:
            # K=4 fp16 hi/lo rank-2 operands: e = (t_hi+t_lo)+(s_hi+s_lo)
            t4_sb = cpool.tile([4, n], F16, tag="t4")   # t_hi, t_lo, 1, 1
            s4_sb = cpool.tile([4, r], F16, tag="s4")   # 1, 1, s_hi, s_lo
            nc.sync.dma_start(t4_sb[2:4, :], ones2h[:])
            nc.sync.dma_start(s4_sb[0:2, :], ones2h[:, 0:r])

        # ---------- prologue: h = x@W, t = x@w2+b, s = x@w1 ----------
        plpool = tc.alloc_tile_pool(name="prolsb", bufs=1)
        ppool = tc.alloc_tile_pool(name="prolps", bufs=2, space=PSUM)

        xT_sb = plpool.tile([dim, n], F32, tag="xT")
        nc.sync.dma_start(xT_sb[:], xT[:])
        xTb_sb = plpool.tile([dim, r], F32, tag="xTb")
        nc.sync.dma_start(xTb_sb[:], xTb[:])
        w1_sb = plpool.tile([dim, 1], F32, tag="w1")
        nc.sync.dma_start(w1_sb[:], w1[:])
        b128_sb = plpool.tile([PJ, 1], F32, tag="b128")
        nc.sync.dma_start(b128_sb[:], b128[:])
        # Fused prologue: one matmul per j-tile with rhs = [W | w2]
        # yields the h tile AND the t column (t[j] = x[j]@w2) for free;
        # t lands column-major as t128[p, jt] = t[jt*128 + p].
        Wx_sb = plpool.tile([dim, dim + 1], F32, tag="Wx")
        nc.sync.dma_start(Wx_sb[:, 0:dim], W[:])
        nc.sync.dma_start(Wx_sb[:, dim:dim + 1], w2[:])
        t128 = plpool.tile([PJ, njt], F32, tag="t128")
        for jt in range(njt):
            hp = ppool.tile([PJ, dim + 1], F32, tag="hp", name=f"hp{jt}")
            nc.tensor.matmul(hp[:], xT_sb[:, jt * PJ:(jt + 1) * PJ], Wx_sb[:])
            nc.vector.tensor_copy(h_sb[:, jt, 0:dim], hp[:, 0:dim])
            nc.vector.tensor_copy(t128[:, jt:jt + 1], hp[:, dim:dim + 1])
        s128 = plpool.tile([PJ, nib], F32, tag="s128")
        for c in range(nib):
            sp = ppool.tile([PJ, 1], F32, tag="sp", name=f"sp{c}")
            nc.tensor.matmul(sp[:], xTb_sb[:, c * PJ:(c + 1) * PJ], w1_sb[:])
            nc.vector.tensor_copy(s128[:, c:c + 1], sp[:])

        def hilo(r128, hl_dram, dst_rows, pfx, bias=None, dst2=None):
            # hi/lo fp16 split on all 128 DVE lanes; j = c*128 + p
            hi = plpool.tile(list(r128.shape), F16, tag=f"{pfx}hi")
            lo = plpool.tile(list(r128.shape), F16, tag=f"{pfx}lo")
            if bias is None:
                nc.vector.tensor_copy(hi[:], r128[:])
                nc.vector.scalar_tensor_tensor(
                    lo[:], r128[:], 1.0, hi[:], ALU.bypass, ALU.subtract)
            else:
                nc.vector.tensor_scalar_add(hi[:], r128[:], bias)
                nc.vector.scalar_tensor_tensor(
                    lo[:], r128[:], bias, hi[:], ALU.add, ALU.subtract)
            nc.sync.dma_start(
                hl_dram[0:1, :].rearrange("o (c p) -> (o p) c", p=PJ), hi[:])
            nc.sync.dma_start(
                hl_dram[1:2, :].rearrange("o (c p) -> (o p) c", p=PJ), lo[:])
            if dst2 is None:
                nc.sync.dma_start(dst_rows, hl_dram[:])
            else:
                for dst, src in dst2:
                    nc.sync.dma_start(dst, src)

        if pack2:
            # split thl by j-tile parity into the two PE row strips
            thl_par = thl_dram[:].rearrange("o (q t c) -> t o q c",
                                            t=2, c=PJ)
            t4e = t4_sb[0:2, :].rearrange("o (q c) -> o q c", c=PJ)
            t4o = t4_sb[32:34, :].rearrange("o (q c) -> o q c", c=PJ)
            hilo(t128[:], thl_dram, t4_sb[0:2, :], "t", bias=b128_sb[:],
                 dst2=[(t4e, thl_par[0]), (t4o, thl_par[1])])
            hilo(s128[:], shl_dram, s4_sb[2:4, :], "s",
                 dst2=[(s4_sb[2:4, :], shl_dram[:]),
                       (s4_sb[34:36, :], shl_dram[:])])
        else:
            hilo(t128[:], thl_dram, t4_sb[0:2, :], "t", bias=b128_sb[:])
            hilo(s128[:], shl_dram, s4_sb[2:4, :], "s")

        ppool.release()
        plpool.release()

        # ---------- main-loop pools ----------
        dpool = tc.alloc_tile_pool(name="dstream", bufs=dbufs)
        wpool = tc.alloc_tile_pool(name="work", bufs=2)
        upool = tc.alloc_tile_pool(name="uhold", bufs=ubufs)
        opool = tc.alloc_tile_pool(name="epi", bufs=4)
        accpool = tc.alloc_tile_pool(name="acc", bufs=1, space=PSUM)
        epool = tc.alloc_tile_pool(name="eps", bufs=ebufs, space=PSUM)

        nA = ngrp - dvp
        for rep in range(repeat):
            ia = 0  # index over flowA (ACT-prelu) groups, for pooladd
            acc = [accpool.tile([PJ, dim + 1], F32, tag=f"acc{ib}",
                                name=f"acc{rep}_{ib}")
                   for ib in range(nib)] if "nomm" not in abl else None
            for g in range(ngrp):
                lt = None
                if "nodma" not in abl:
                    if g % dsup == 0:
                        # one 1 MiB DMA covers dsup elementwise groups
                        npair = dsup * grp // 2
                        ltb = dpool.tile([PJ, npair, 2 * r], F16, tag="lt",
                                         name=f"lt{rep}_{g}")
                        q0 = g * grp // 2
                        nc.sync.dma_start(
                            ltb[:],
                            Lm[q0 * PJ:(q0 + npair) * PJ, :]
                            .rearrange("(q p) f -> p q f", p=PJ))
                    p0 = (g % dsup) * grp // 2
                    lt = (ltb[:, p0:p0 + grp // 2, :]
                          .rearrange("p a i -> p (a i)"))

                e_ps = epool.tile([PJ, grp, r], F32, tag="e",
                                  name=f"e{rep}_{g}")
                if pack2:
                    for a in range(0, grp, 2):
                        q = (g * grp + a) // 2
                        cs = slice(q * PJ, (q + 1) * PJ)
                        nc.tensor.matmul(e_ps[:, a, :], t4_sb[0:4, cs],
                                         s4_sb[0:4, :], tile_position=(0, 0))
                        nc.tensor.matmul(e_ps[:, a + 1, :], t4_sb[32:36, cs],
                                         s4_sb[32:36, :],
                                         tile_position=(32, 0))
                else:
                    for a in range(grp):
                        jt = g * grp + a
                        nc.tensor.matmul(e_ps[:, a, :],
                                         t4_sb[:, jt * PJ:(jt + 1) * PJ],
                                         s4_sb[:])
                epf = e_ps[:].rearrange("p a i -> p (a i)")

                # Ablations substitute inputs rather than skip consumers so
                # every tile that is read stays written (Tile requires it).
                dve_leaky = (g % ngrp) * dvp // ngrp != \
                    ((g % ngrp) + 1) * dvp // ngrp  # spread dvp-of-ngrp
                src0 = None
                if "noprelu" not in abl:
                    e2 = wpool.tile([PJ, fr], F16, tag="e2",
                                    name=f"e2{rep}_{g}")
                    if dve_leaky:
                        # leaky on DVE with ONE PSUM pass: drain 0.2e,
                        # then leaky = max(5*(0.2e), 0.2e) on SBUF fp16
                        # (both operands 16-bit -> 2x perf mode)
                        rt = wpool.tile([PJ, fr], F16, tag="rt",
                                        name=f"rt{rep}_{g}")
                        nc.vector.tensor_scalar_mul(rt[:], epf, 0.2)
                        nc.vector.scalar_tensor_tensor(
                            e2[:], rt[:], 5.0, rt[:], ALU.mult, ALU.max)
                    else:
                        # single ACT Prelu (HW-verified alpha semantics)
                        nc.scalar.activation(e2[:], epf, AF.Prelu,
                                             alpha=NEG_SLOPE)
                    src0 = e2[:]
                if src0 is None:
                    src0 = lt
                assert src0 is not None, "noprelu+nodma unsupported"
                src1 = lt if lt is not None else src0
                esrc = src0
                if "noadd" not in abl:
                    if fuse2:
                        if g % 2 == 0:
                            gt2 = wpool.tile([PJ, 2, fr], F16, tag="g2",
                                             name=f"g2{rep}_{g}")
                        gt = gt2[:, g % 2, :]
                    else:
                        gt = wpool.tile([PJ, fr], F16, tag="g",
                                        name=f"g{rep}_{g}")[:]
                    # offload some flowA adds to the otherwise-idle GPSIMD
                    on_pool = False
                    if not dve_leaky and nA > 0:
                        on_pool = ia * pooladd // nA != \
                            (ia + 1) * pooladd // nA
                        ia += 1
                    eng = nc.gpsimd if on_pool else nc.vector
                    cw = fr // chop
                    for cc in range(chop):
                        s = slice(cc * cw, (cc + 1) * cw)
                        eng.tensor_add(gt[:, s], src0[:, s], src1[:, s])
                    esrc = gt
                def contraction(g_, ut_):
                    for a in range(grp):
                        jt = g_ * grp + a
                        for ib in range(nib):
                            nc.tensor.matmul(
                                acc[ib][:],
                                ut_[:, a, ib * PJ:(ib + 1) * PJ],
                                h_sb[:, jt, :],
                                start=(jt == 0), stop=(jt == njt - 1))

                if "noexp" not in abl:
                    if fuse2:
                        # one Exp per PAIR of groups (FD=2*fr amortizes
                        # the ~220-cycle per-op ACT overhead).  The even
                        # group's contraction is DEFERRED until after the
                        # fused exp so no reader precedes the writer.
                        if g % 2 == 0:
                            ut2 = upool.tile([PJ, 2, grp, r], F16, tag="u",
                                             name=f"u{rep}_{g}")
                        else:
                            nc.scalar.activation(
                                ut2[:].rearrange("p q a i -> p (q a i)"),
                                gt2[:].rearrange("p q f -> p (q f)"), AF.Exp)
                            if "nomm" not in abl:
                                contraction(g - 1, ut2[:, 0])
                                contraction(g, ut2[:, 1])
                    else:
                        ut = upool.tile([PJ, grp, r], F16, tag="u",
                                        name=f"u{rep}_{g}")
                        nc.scalar.activation(
                            ut[:].rearrange("p a i -> p (a i)"), esrc, AF.Exp)
                        if "nomm" not in abl:
                            contraction(g, ut[:])
                elif "nomm" not in abl:
                    for a in range(grp):
                        jt = g * grp + a
                        for ib in range(nib):
                            nc.tensor.matmul(
                                acc[ib][:],
                                esrc[:, a * r + ib * PJ:a * r + (ib + 1) * PJ],
                                h_sb[:, jt, :],
                                start=(jt == 0), stop=(jt == njt - 1))

            # ---------- epilogue: out = num / rowsum ----------
            # (reference's +1e-9 is ~1e-12 relative here: rowsums are the
            # UNnormalized exp-sums, O(100), so the epsilon is dropped)
            if "nomm" not in abl:
                ot4 = opool.tile([PJ, nib, dim], F32, tag="ot",
                                 name=f"ot{rep}")
                for ib in range(nib):
                    rec = opool.tile([PJ, 1], F32, tag="rec",
                                     name=f"rec{rep}_{ib}")
                    nc.vector.reciprocal(rec[:], acc[ib][:, dim:dim + 1])
                    nc.vector.tensor_scalar_mul(ot4[:, ib, :],
                                                acc[ib][:, 0:dim], rec[:])
                nc.sync.dma_start(
                    out[:].rearrange("(q p) d -> p q d", p=PJ), ot4[:])

        epool.release()
        accpool.release()
        opool.release()
        upool.release()
        wpool.release()
        dpool.release()
        cpool.release()

    nc.compile()
    return nc


_NC_CACHE = {}


def _get_nc(**kw):
    key = tuple(sorted((k, v) for k, v in kw.items()))
    if key not in _NC_CACHE:
        _NC_CACHE[key] = build_nc(**kw)
    return _NC_CACHE[key]


def host_prep(x, adj, dist_mat, angle_mat, W, attn_w, attn_b, n=N, dim=DIM,
              ncores=NCORES):
    """Shard + marshal inputs into the per-core layout."""
    x = np.ascontiguousarray(np.asarray(x, dtype=np.float32))
    adj = np.asarray(adj)
    dist_mat = np.asarray(dist_mat, dtype=np.float32)
    angle_mat = np.asarray(angle_mat, dtype=np.float32)
    W = np.ascontiguousarray(np.asarray(W, dtype=np.float32))
    attn_w = np.asarray(attn_w, dtype=np.float32)
    attn_b = np.asarray(attn_b, dtype=np.float32)

    r = n // ncores
    xT = np.ascontiguousarray(x.T)                      # [dim, n]
    w1 = np.ascontiguousarray((W @ attn_w[:dim]).reshape(dim, 1))
    w2 = np.ascontiguousarray((W @ attn_w[dim:]).reshape(dim, 1))
    bb = float(attn_b.reshape(-1)[0])

    # Fold the physics rescale + adjacency mask into one log-domain
    # matrix: w = exp(leaky(e) + L); masked entries underflow to 0.
    cosw = np.clip(np.cos(angle_mat), 0.0, None) + np.float32(1e-6)
    L = np.where(adj != 0, -dist_mat + np.log(cosw),
                 np.float32(MASKL)).astype(np.float32)

    in_maps = []
    njt = n // PJ
    for c in range(ncores):
        sl = slice(c * r, (c + 1) * r)
        LT = L[sl].T.astype(np.float16)                 # [n, r]
        # pair-granularity marshal: [njt/2, 2, 128, r] -> [njt/2, 128, 2, r]
        Lm = np.ascontiguousarray(
            LT.reshape(njt // 2, 2, PJ, r).transpose(0, 2, 1, 3)
            .reshape((njt // 2) * PJ, 2 * r))
        in_maps.append({
            "ones2h": np.ones((2, n), dtype=np.float16),
            "b128": np.full((PJ, 1), bb, dtype=np.float32),
            "xT": xT,
            "xTb": np.ascontiguousarray(xT[:, sl]),
            "W": W,
            "w1": w1,
            "w2": w2,
            "Lm": Lm,
        })
    return in_maps


def kernel(x, adj, dist_mat, angle_mat, W, attn_w, attn_b):
    from concourse.bass_utils import run_bass_kernel_spmd

    nc = _get_nc()
    in_maps = host_prep(x, adj, dist_mat, angle_mat, W, attn_w, attn_b)
    last_err = None
    for attempt in range(3):
        try:
            res = run_bass_kernel_spmd(nc, in_maps,
                                       core_ids=list(range(NCORES)))
            return np.concatenate(
                [res.results[c]["out"] for c in range(NCORES)], axis=0)
        except Exception as ex:  # axon terminals occasionally come up wedged
            last_err = ex
            try:
                import jax
                jax.clear_caches()
                jax._src.api.clear_backends()
            except Exception:
                pass
    raise last_err
